# revision 6
# baseline (speedup 1.0000x reference)
"""LSTM cell kernel for Trainium2, 8 NeuronCores, data-parallel over batch.

Math: stacked = x @ Wx + bx + prevh @ Wh
      i,f,o,g = split(stacked, 4, axis=1); i,f,o = sigmoid; g = tanh
      nextc = prevc*f + g*i ; nexth = tanh(nextc)*o

Device strategy (per core, batch shard of 1024 rows):
  - Host pre-concats [x|prevh] and [Wx;Wh] into one K=2048 contraction and
    quantizes both sides to scaled fp8 e4m3 (x*16, W*4096).  Matmuls run in
    MatmulPerfMode.DoubleRow: each instruction contracts 256 k-rows
    (2 x 128 partitions) at 0.5 cycles per output column - 4x the bf16
    row rate under the cost model.
  - Mixed per-gate precision keeps rel-err under the 2e-2 gate: i/f/o use a
    single fp8 pass; the tanh gate g (largest error sensitivity) accumulates
    three passes in PSUM: x8@W8 + rx8@W8 + x8@RW8, where rx8/RW8 are fp8
    quantization residuals at the same scale (effective ~bf16 accuracy).
  - Weight columns reordered into per-gate 128-col blocks grouped by state
    block j with device gate order (i, f, g, o); one PSUM tile is one gate
    half for one state block.  The 1/65536 scale is folded into the fused
    ACT eviction (func(psum*scale + bias)).
  - prevc is loaded and nexth/nextc are stored as bf16 (negligible error,
    halves traffic on the serialized DMA device).  Elementwise combine in
    [state, batch] layout; outputs un-transposed and upcast on host.
"""

import os
import sys

sys.path.insert(0, "/opt/trn_rl_repo")
# v2 ASAP tile scheduler: measurably tighter schedule than the legacy flow
os.environ.setdefault("TILE_SCHEDULER", "asap")

import numpy as np

BATCH = 8192
DIM = 1024  # INPUT_DIM == STATE_DIM
K = 2 * DIM  # stacked contraction [x|prevh]
NCORES = 8
B_LOC = BATCH // NCORES  # 1024
N_KT = K // 128  # 16 k-tiles of 128
N_K2 = K // 256  # 8 DoubleRow k-steps of 256
N_GT = 4 * DIM // 128  # 32 gate-block tiles
N_J = DIM // 128  # 8 state blocks

SX = 16.0  # fp8 scale for activations
SW = 4096.0  # fp8 scale for weights
ISCALE = 1.0 / (SX * SW)

_CACHED = {}


def _build_program():
    import ml_dtypes  # noqa: F401
    from concourse import bass, tile
    from concourse.bass import mybir

    f8 = mybir.dt.float8e4
    bf16 = mybir.dt.bfloat16
    f32 = mybir.dt.float32
    AF = mybir.ActivationFunctionType
    DR = mybir.MatmulPerfMode.DoubleRow

    nc = bass.Bass("TRN2", target_bir_lowering=False)
    x8_d = nc.dram_tensor("x8", [K, B_LOC], f8, kind="ExternalInput")
    rx8_d = nc.dram_tensor("rx8", [K, B_LOC], f8, kind="ExternalInput")
    w8_d = nc.dram_tensor("w8", [N_GT, 128, N_K2, 2, 128], f8, kind="ExternalInput")
    rw8_d = nc.dram_tensor("rw8", [N_J, 128, N_K2, 2, 128], f8, kind="ExternalInput")
    bias_d = nc.dram_tensor("bias", [128, N_GT], f32, kind="ExternalInput")
    pcT_d = nc.dram_tensor("pcT", [DIM, B_LOC], bf16, kind="ExternalInput")
    hT_d = nc.dram_tensor("hT", [DIM, B_LOC], bf16, kind="ExternalOutput")
    cT_d = nc.dram_tensor("cT", [DIM, B_LOC], bf16, kind="ExternalOutput")

    with tile.TileContext(nc) as tc:
        with (
            tc.tile_pool(name="const", bufs=1) as const_pool,
            tc.tile_pool(name="wp", bufs=10) as w_pool,
            tc.tile_pool(name="rwp", bufs=3) as rw_pool,
            tc.tile_pool(name="pc", bufs=3) as pc_pool,
            tc.tile_pool(name="gates", bufs=10) as g_pool,
            tc.tile_pool(name="outs", bufs=4) as out_pool,
            tc.tile_pool(name="psum", bufs=8, space="PSUM") as psum_pool,
        ):
            # resident activations: x8 + residual, [128, kt, 1024] fp8,
            # 16KB/partition each.  A DoubleRow moving slice is
            # [:, 2*k2:2*k2+2, h*512:...] -> AP [128, 2, 512].
            xh8_sb = const_pool.tile([128, N_KT, B_LOC], f8)
            rx8_sb = const_pool.tile([128, N_KT, B_LOC], f8)
            bias_sb = const_pool.tile([128, N_GT], f32)

            # dummy matmuls first in PE program order: warm the PE HAM clock
            # gate (3us busy window) while the startup DMAs stream
            warm_sb = const_pool.tile([1, 128], f8)
            nc.gpsimd.memset(warm_sb[:], 0.0)
            warm_ps = psum_pool.tile([128, 512], f32, tag="ps")
            for _ in range(24):
                nc.tensor.matmul(
                    warm_ps[:, 0:64],
                    warm_sb[:, 0:128],
                    warm_sb[:, 0:64],
                    start=True,
                    stop=True,
                )

            def load_x(kt):
                nc.sync.dma_start(xh8_sb[:, kt], x8_d[kt * 128 : (kt + 1) * 128, :])

            def load_rx(kt):
                nc.scalar.dma_start(rx8_sb[:, kt], rx8_d[kt * 128 : (kt + 1) * 128, :])

            w_tiles = {}

            def load_w(gt):
                w_sb = w_pool.tile([128, N_K2, 2, 128], f8, tag="w")
                nc.sync.dma_start(w_sb[:], w8_d[gt])
                w_tiles[gt] = w_sb

            rw_tiles = {}

            def load_rw(j):
                rw_sb = rw_pool.tile([128, N_K2, 2, 128], f8, tag="rw")
                nc.scalar.dma_start(rw_sb[:], rw8_d[j])
                rw_tiles[j] = rw_sb

            # startup DMAs: j0's weights and the x stream on sync, residuals
            # and bias on the scalar hwdge queue
            load_w(0)
            load_x(0)
            load_x(1)
            load_w(1)
            nc.scalar.dma_start(bias_sb[:], bias_d[:])
            load_x(2)
            load_x(3)
            load_w(2)
            for kt in range(4, 8):
                load_x(kt)
            load_w(3)
            for kt in range(8, N_KT):
                load_x(kt)
            for kt in range(N_KT):
                load_rx(kt)
            load_rw(0)

            for j in range(N_J):
                last_j = j == N_J - 1
                # prefetch next block's weights so mid-kernel matmuls never
                # wait on the (serialized) DMA device
                if not last_j:
                    for gate in range(4):
                        load_w((j + 1) * 4 + gate)
                    load_rw(j + 1)
                pc_sb = None

                def gate_passes(gate):
                    """(stationary tile, moving tile) per accumulation pass."""
                    gt = j * 4 + gate
                    if gate == 2:
                        w_sb = w_tiles.pop(gt)
                        return [
                            (w_sb, xh8_sb),
                            (w_sb, rx8_sb),
                            (rw_tiles.pop(j), xh8_sb),
                        ]
                    return [(w_tiles.pop(gt), xh8_sb)]

                def run_gates(gates, evict=True):
                    """Issue DoubleRow matmuls for one or more gates,
                    k-interleaved when len>1 (keeps PE fed while x8 tiles
                    still stream in during j=0)."""
                    tiles = {}
                    steps = {}
                    for gate in gates:
                        passes = gate_passes(gate)
                        ps0 = psum_pool.tile([128, 512], f32, tag="ps")
                        ps1 = psum_pool.tile([128, 512], f32, tag="ps")
                        tiles[gate] = (ps0, ps1)
                        # flatten passes into virtual k-steps
                        vk = []
                        for w_sb, m_sb in passes:
                            for k2 in range(N_K2):
                                vk.append((w_sb, m_sb, k2))
                        steps[gate] = vk
                    n_steps = max(len(s) for s in steps.values())
                    for vi in range(n_steps):
                        for gate in gates:
                            vk = steps[gate]
                            if vi >= len(vk):
                                continue
                            w_sb, m_sb, k2 = vk[vi]
                            first, last = vi == 0, vi == len(vk) - 1
                            ps0, ps1 = tiles[gate]
                            lhsT = w_sb[:, k2]
                            nc.tensor.matmul(
                                ps0[:],
                                lhsT,
                                m_sb[:, 2 * k2 : 2 * k2 + 2, 0:512],
                                start=first,
                                stop=last,
                                perf_mode=DR,
                            )
                            nc.tensor.matmul(
                                ps1[:],
                                lhsT,
                                m_sb[:, 2 * k2 : 2 * k2 + 2, 512:B_LOC],
                                start=first,
                                stop=last,
                                perf_mode=DR,
                            )
                    if not evict:
                        return tiles
                    for gate in gates:
                        gt = j * 4 + gate
                        ps0, ps1 = tiles[gate]
                        g_sb = g_pool.tile([128, B_LOC], f32, tag="g")
                        func = AF.Tanh if gate == 2 else AF.Sigmoid
                        nc.scalar.activation(
                            g_sb[:, 0:512],
                            ps0[:],
                            func,
                            bias=bias_sb[:, gt : gt + 1],
                            scale=ISCALE,
                        )
                        nc.scalar.activation(
                            g_sb[:, 512:B_LOC],
                            ps1[:],
                            func,
                            bias=bias_sb[:, gt : gt + 1],
                            scale=ISCALE,
                        )
                        done[gate] = g_sb
                    return tiles

                done = {}
                # for the last block run the heavy 3-pass g gate first so the
                # c/h chain overlaps the remaining matmuls instead of tailing
                if j == 0:
                    groups = [(0, 1), (2,)]
                elif last_j:
                    groups = [(2,), (0,), (1,)]
                else:
                    groups = [(0,), (1,), (2,)]

                for gates in groups:
                    run_gates(gates)
                    if pc_sb is None:
                        # deferred so the transfer doesn't compete with the
                        # startup x8 stream; needed only at the c-chain
                        pc_sb = pc_pool.tile([128, B_LOC], bf16)
                        nc.scalar.dma_start(
                            pc_sb[:], pcT_d[j * 128 : (j + 1) * 128, :]
                        )

                i_t, f_t, g_t = done[0], done[1], done[2]
                if not last_j:
                    # i, f, g ready: compute c and tanh(c) while o's
                    # matmuls run
                    c16 = out_pool.tile([128, B_LOC], bf16, tag="c")
                    tmp = out_pool.tile([128, B_LOC], f32, tag="tmp")
                    c0 = out_pool.tile([128, B_LOC], f32, tag="c0")
                    nc.vector.tensor_mul(out=tmp[:], in0=i_t[:], in1=g_t[:])
                    nc.vector.tensor_mul(out=c0[:], in0=f_t[:], in1=pc_sb[:])
                    nc.vector.tensor_add(out=c16[:], in0=c0[:], in1=tmp[:])
                    nc.scalar.dma_start(cT_d[j * 128 : (j + 1) * 128, :], c16[:])
                    h_sb = out_pool.tile([128, B_LOC], f32, tag="h")
                    nc.scalar.activation(h_sb[:], c16[:], AF.Tanh)

                    run_gates((3,))
                    o_t = done[3]
                    h16 = g_pool.tile([128, B_LOC], bf16, tag="h16")
                    nc.vector.tensor_mul(out=h16[:], in0=h_sb[:], in1=o_t[:])
                    nc.scalar.dma_start(hT_d[j * 128 : (j + 1) * 128, :], h16[:])
                else:
                    # final state block: o matmuls run while the c chain is
                    # computed in halves; then evict o and form h in quarter
                    # chunks so the post-last-matmul serial chain is short
                    tiles = run_gates((3,), evict=False)
                    c16 = out_pool.tile([128, B_LOC], bf16, tag="c")
                    tmp = out_pool.tile([128, B_LOC], f32, tag="tmp")
                    c0 = out_pool.tile([128, B_LOC], f32, tag="c0")
                    h_sb = out_pool.tile([128, B_LOC], f32, tag="h")
                    for hb in range(2):
                        hs = slice(hb * 512, (hb + 1) * 512)
                        nc.vector.tensor_mul(
                            out=tmp[:, hs], in0=i_t[:, hs], in1=g_t[:, hs]
                        )
                        nc.vector.tensor_mul(
                            out=c0[:, hs], in0=f_t[:, hs], in1=pc_sb[:, hs]
                        )
                        nc.vector.tensor_add(
                            out=c16[:, hs], in0=c0[:, hs], in1=tmp[:, hs]
                        )
                        nc.scalar.dma_start(
                            cT_d[j * 128 : (j + 1) * 128, hs], c16[:, hs]
                        )
                        nc.scalar.activation(h_sb[:, hs], c16[:, hs], AF.Tanh)
                    ps0, ps1 = tiles[3]
                    gt = j * 4 + 3
                    o_sb = g_pool.tile([128, B_LOC], f32, tag="g")
                    h16 = g_pool.tile([128, B_LOC], bf16, tag="h16")
                    for cb in range(4):
                        ps = ps0 if cb < 2 else ps1
                        pslice = slice((cb % 2) * 256, (cb % 2) * 256 + 256)
                        bslice = slice(cb * 256, (cb + 1) * 256)
                        nc.scalar.activation(
                            o_sb[:, bslice],
                            ps[:, pslice],
                            AF.Sigmoid,
                            bias=bias_sb[:, gt : gt + 1],
                            scale=ISCALE,
                        )
                        nc.vector.tensor_mul(
                            out=h16[:, bslice],
                            in0=o_sb[:, bslice],
                            in1=h_sb[:, bslice],
                        )
                        nc.sync.dma_start(
                            hT_d[j * 128 : (j + 1) * 128, bslice], h16[:, bslice]
                        )

    nc.finalize()
    _install_wait_splitter(nc)
    return nc


def _split_multiwaits(mod: dict) -> dict:
    """This container's walrus encodes at most ONE sync wait per instruction
    (setupSyncWait raises 'Too many sync wait commands'), while Tile emits
    several. Move excess waits onto standalone single-wait EventSemaphore
    instructions inserted just before, on the same engine. All excess waits
    must be monotone (sem-ge-imm) for the serialization to be equivalent.
    """
    for fn in mod.get("functions", []):
        for blk in fn.get("blocks", []):
            insts = blk.get("instructions") or []
            out = []
            for inst in insts:
                si = inst.get("sync_info")
                waits = (si or {}).get("on_wait") or []
                if len(waits) > 1:
                    keep, extra = [], []
                    # keep non-monotone waits (if any) on the instruction
                    for w in waits:
                        (extra if w.get("wait_mode") == "sem-ge-imm" else keep).append(w)
                    if not keep:
                        keep.append(extra.pop())
                    for n, w in enumerate(extra):
                        out.append(
                            {
                                "name": f"{inst['name']}_sw{n}",
                                "opcode": "EventSemaphore",
                                "engine": inst["engine"],
                                "debug": inst.get("debug", 0),
                                "sync_info": {"on_wait": [w], "on_update": []},
                            }
                        )
                    si["on_wait"] = keep
                out.append(inst)
            blk["instructions"] = out
    return mod


def _install_wait_splitter(nc):
    import json as _json

    orig = nc.to_json_bytes

    def patched():
        mod = _json.loads(orig())
        return _json.dumps(_split_multiwaits(mod)).encode()

    nc.to_json_bytes = patched


def _quant(a, scale):
    """Scaled e4m3 value + same-scale residual (both as fp8)."""
    import ml_dtypes

    e4 = ml_dtypes.float8_e4m3
    s = np.clip(a * scale, -240.0, 240.0).astype(np.float32)
    q = s.astype(e4)
    r = np.clip(s - q.astype(np.float32), -240.0, 240.0).astype(e4)
    return q, r


def _prep_shared(Wx, bx, Wh):
    W = np.concatenate([Wx, Wh], axis=0)  # [K, 4*DIM]
    # columns gate*DIM + j*128 + c -> (j*4 + pos)*128 + c with device gate
    # order (i, f, g, o) within each state block j
    W_re = (
        W.reshape(K, 4, N_J, 128)[:, [0, 1, 3, 2]]
        .transpose(0, 2, 1, 3)
        .reshape(K, 4 * DIM)
    )
    W8, RW8 = _quant(W_re, SW)

    def dev_layout(A):  # [K, 4*DIM] fp8 -> [N_GT, 128, N_K2, 2, 128]
        return np.ascontiguousarray(
            A.reshape(N_K2, 2, 128, N_GT, 128).transpose(3, 2, 0, 1, 4)
        )

    w8_dev = dev_layout(W8)
    rw8_dev = np.ascontiguousarray(dev_layout(RW8)[2::4])  # g-gate tiles only
    b_re = bx.reshape(4, N_J, 128)[[0, 1, 3, 2]].transpose(1, 0, 2).reshape(4 * DIM)
    bias_dev = np.ascontiguousarray(b_re.reshape(N_GT, 128).T, dtype=np.float32)
    return w8_dev, rw8_dev, bias_dev


def kernel(x, prevh, prevc, Wx, bx, Wh):
    import ml_dtypes
    from concourse import bass_utils

    bf16 = ml_dtypes.bfloat16
    x, prevh, prevc, Wx, bx, Wh = (
        np.asarray(a, dtype=np.float32) for a in (x, prevh, prevc, Wx, bx, Wh)
    )

    if "nc" not in _CACHED:
        _CACHED["nc"] = _build_program()
    nc = _CACHED["nc"]

    w8_dev, rw8_dev, bias_dev = _prep_shared(Wx, bx, Wh)

    in_maps = []
    for c in range(NCORES):
        rows = slice(c * B_LOC, (c + 1) * B_LOC)
        xh = np.concatenate([x[rows], prevh[rows]], axis=1)  # [B_LOC, K]
        x8, rx8 = _quant(np.ascontiguousarray(xh.T), SX)  # [K, B_LOC]
        pcT = np.ascontiguousarray(prevc[rows].T.astype(bf16))
        in_maps.append(
            {
                "x8": x8,
                "rx8": rx8,
                "w8": w8_dev,
                "rw8": rw8_dev,
                "bias": bias_dev,
                "pcT": pcT,
            }
        )
    _CACHED["in_maps"] = in_maps

    res = bass_utils.run_bass_kernel_spmd(nc, in_maps, core_ids=list(range(NCORES)))

    nexth = np.empty((BATCH, DIM), np.float32)
    nextc = np.empty((BATCH, DIM), np.float32)
    for c in range(NCORES):
        rows = slice(c * B_LOC, (c + 1) * B_LOC)
        nexth[rows] = np.asarray(res.results[c]["hT"]).astype(np.float32).T
        nextc[rows] = np.asarray(res.results[c]["cT"]).astype(np.float32).T
    return nexth, nextc


if __name__ == "__main__":
    rng = np.random.default_rng(0)
    inputs = {
        "x": rng.standard_normal((BATCH, DIM), np.float32),
        "prevh": rng.standard_normal((BATCH, DIM), np.float32),
        "prevc": rng.standard_normal((BATCH, DIM), np.float32),
        "Wx": (rng.random((DIM, 4 * DIM), np.float32) - 0.5) / 16,
        "bx": (rng.random(4 * DIM, np.float32) - 0.5) / 16,
        "Wh": (rng.random((DIM, 4 * DIM), np.float32) - 0.5) / 16,
    }
    h, c = kernel(**inputs)
    print("ok", h.shape, c.shape, h.dtype)


# revision 8
# speedup vs baseline: 1.0674x; 1.0674x over previous
"""LSTM cell kernel for Trainium2, 8 NeuronCores, data-parallel over batch.

Math: stacked = x @ Wx + bx + prevh @ Wh
      i,f,o,g = split(stacked, 4, axis=1); i,f,o = sigmoid; g = tanh
      nextc = prevc*f + g*i ; nexth = tanh(nextc)*o

Device strategy (per core, batch shard of 1024 rows):
  - Host pre-concats [x|prevh] and [Wx;Wh] into one K=2048 contraction and
    quantizes both sides to scaled fp8 e4m3 (x*16, W*4096).  Matmuls run in
    MatmulPerfMode.DoubleRow: each instruction contracts 256 k-rows
    (2 x 128 partitions) at 0.5 cycles per output column - 4x the bf16
    row rate under the cost model.
  - Mixed per-gate precision keeps rel-err under the 2e-2 gate: i/f/o use a
    single fp8 pass; the tanh gate g (largest error sensitivity) accumulates
    three passes in PSUM: x8@W8 + rx8@W8 + x8@RW8, where rx8/RW8 are fp8
    quantization residuals at the same scale (effective ~bf16 accuracy).
  - DMA instruction count is minimized (the HWDGE device serializes at
    ~630ns/DMA): activations stream in 4 quarter-DMAs per tensor and each
    state block's 5 weight tiles (i,f,g,o,rw) arrive in one superblock DMA.
  - Per state block (device gate order i,f,g,o): evictions are emitted
    eviction-first so the in-order ACT queue frees PSUM banks promptly; the
    c/h elementwise chain runs after o's matmuls are issued and overlaps the
    next block on other engines.  j0 interleaves all four gates k-wise to
    match the startup x-stream rate; the last block runs g first and
    pipelines its epilogue in quarter chunks.
  - prevc loads and nexth/nextc stores are bf16 (negligible error, halves
    DMA traffic).  The 1/65536 fp8 scale is folded into the fused ACT
    eviction (func(psum*scale + bias)).  Outputs un-transposed on host.
"""

import os
import sys

sys.path.insert(0, "/opt/trn_rl_repo")
# v2 ASAP tile scheduler: measurably tighter schedule than the legacy flow
os.environ.setdefault("TILE_SCHEDULER", "asap")

import numpy as np

BATCH = 8192
DIM = 1024  # INPUT_DIM == STATE_DIM
K = 2 * DIM  # stacked contraction [x|prevh]
NCORES = 8
B_LOC = BATCH // NCORES  # 1024
N_KT = K // 128  # 16 k-tiles of 128
N_K2 = K // 256  # 8 DoubleRow k-steps of 256
N_GT = 4 * DIM // 128  # 32 gate-block tiles
N_J = DIM // 128  # 8 state blocks

SX = 16.0  # fp8 scale for activations
SW = 4096.0  # fp8 scale for weights
ISCALE = 1.0 / (SX * SW)

_CACHED = {}


def _build_program():
    import ml_dtypes  # noqa: F401
    from concourse import bass, tile
    from concourse.bass import mybir

    f8 = mybir.dt.float8e4
    bf16 = mybir.dt.bfloat16
    f32 = mybir.dt.float32
    AF = mybir.ActivationFunctionType
    DR = mybir.MatmulPerfMode.DoubleRow

    nc = bass.Bass("TRN2", target_bir_lowering=False)
    # activations pre-chunked on host: [partition, kt, col]
    x8_d = nc.dram_tensor("x8", [128, N_KT, B_LOC], f8, kind="ExternalInput")
    rx8_d = nc.dram_tensor("rx8", [128, N_KT, B_LOC], f8, kind="ExternalInput")
    # per-state-block weight superblock: s = 0..3 gates (i,f,g,o), 4 = rw(g)
    w8_d = nc.dram_tensor(
        "w8", [N_J, 128, 5, N_K2, 2, 128], f8, kind="ExternalInput"
    )
    bias_d = nc.dram_tensor("bias", [128, N_GT], f32, kind="ExternalInput")
    pcT_d = nc.dram_tensor("pcT", [DIM, B_LOC], bf16, kind="ExternalInput")
    hT_d = nc.dram_tensor("hT", [DIM, B_LOC], bf16, kind="ExternalOutput")
    cT_d = nc.dram_tensor("cT", [DIM, B_LOC], bf16, kind="ExternalOutput")

    with tile.TileContext(nc) as tc:
        with (
            tc.tile_pool(name="const", bufs=1) as const_pool,
            tc.tile_pool(name="wp", bufs=3) as w_pool,
            tc.tile_pool(name="pc", bufs=3) as pc_pool,
            tc.tile_pool(name="gates", bufs=10) as g_pool,
            tc.tile_pool(name="outs", bufs=4) as out_pool,
            tc.tile_pool(name="psum", bufs=8, space="PSUM") as psum_pool,
        ):
            # resident activations: x8 + residual, [128, kt, 1024] fp8,
            # 16KB/partition each.  A DoubleRow moving slice is
            # [:, 2*k2:2*k2+2, h*512:...] -> AP [128, 2, 512].
            xh8_sb = const_pool.tile([128, N_KT, B_LOC], f8)
            rx8_sb = const_pool.tile([128, N_KT, B_LOC], f8)
            bias_sb = const_pool.tile([128, N_GT], f32)

            # dummy matmuls first in PE program order: warm the PE HAM clock
            # gate (3us busy window) while the startup DMAs stream
            warm_sb = const_pool.tile([1, 128], f8)
            nc.gpsimd.memset(warm_sb[:], 0.0)
            warm_ps = psum_pool.tile([128, 512], f32, tag="ps")
            for _ in range(30):
                nc.tensor.matmul(
                    warm_ps[:, 0:64],
                    warm_sb[:, 0:128],
                    warm_sb[:, 0:64],
                    start=True,
                    stop=True,
                )

            w_tiles = {}

            def load_w(j):
                w_sb = w_pool.tile([128, 5, N_K2, 2, 128], f8, tag="w")
                nc.sync.dma_start(w_sb[:], w8_d[j])
                w_tiles[j] = w_sb

            # startup: j0 weights and the x stream on sync (split small for a
            # fast PE start), residuals and bias on the scalar hwdge queue
            w0_sb = w_pool.tile([128, 5, N_K2, 2, 128], f8, tag="w")
            w_tiles[0] = w0_sb
            nc.sync.dma_start(w0_sb[:, 0:1], w8_d[0][:, 0:1])
            nc.sync.dma_start(xh8_sb[:, 0:2], x8_d[:, 0:2])
            nc.sync.dma_start(w0_sb[:, 1:2], w8_d[0][:, 1:2])
            nc.sync.dma_start(xh8_sb[:, 2:4], x8_d[:, 2:4])
            nc.scalar.dma_start(bias_sb[:], bias_d[:])
            nc.sync.dma_start(w0_sb[:, 2:5], w8_d[0][:, 2:5])
            nc.sync.dma_start(xh8_sb[:, 4:8], x8_d[:, 4:8])
            nc.sync.dma_start(xh8_sb[:, 8:12], x8_d[:, 8:12])
            nc.sync.dma_start(xh8_sb[:, 12:16], x8_d[:, 12:16])
            for q in range(4):
                nc.scalar.dma_start(
                    rx8_sb[:, 4 * q : 4 * q + 4], rx8_d[:, 4 * q : 4 * q + 4]
                )

            for j in range(N_J):
                last_j = j == N_J - 1
                # prefetch next block's weight superblock; load this block's
                # prevc early (no deps, so it never parks the ACT queue)
                if not last_j:
                    load_w(j + 1)
                pc_sb = pc_pool.tile([128, B_LOC], bf16)
                nc.scalar.dma_start(pc_sb[:], pcT_d[j * 128 : (j + 1) * 128, :])
                w_all = w_tiles.pop(j)

                def gate_passes(gate):
                    """(stationary s-index, moving tile) per accumulation
                    pass."""
                    if gate == 2:
                        return [(2, xh8_sb), (2, rx8_sb), (4, xh8_sb)]
                    return [(gate, xh8_sb)]

                def run_gates(gates, evict=True):
                    """Issue DoubleRow matmuls for one or more gates,
                    k-interleaved when len>1, then evict eviction-first so
                    the in-order ACT queue frees PSUM banks promptly."""
                    tiles = {}
                    steps = {}
                    for gate in gates:
                        ps0 = psum_pool.tile([128, 512], f32, tag="ps")
                        ps1 = psum_pool.tile([128, 512], f32, tag="ps")
                        tiles[gate] = (ps0, ps1)
                        vk = []
                        for s, m_sb in gate_passes(gate):
                            for k2 in range(N_K2):
                                vk.append((s, m_sb, k2))
                        steps[gate] = vk
                    n_steps = max(len(s) for s in steps.values())
                    for vi in range(n_steps):
                        for gate in gates:
                            vk = steps[gate]
                            if vi >= len(vk):
                                continue
                            s, m_sb, k2 = vk[vi]
                            first, last = vi == 0, vi == len(vk) - 1
                            ps0, ps1 = tiles[gate]
                            lhsT = w_all[:, s, k2]
                            nc.tensor.matmul(
                                ps0[:],
                                lhsT,
                                m_sb[:, 2 * k2 : 2 * k2 + 2, 0:512],
                                start=first,
                                stop=last,
                                perf_mode=DR,
                            )
                            nc.tensor.matmul(
                                ps1[:],
                                lhsT,
                                m_sb[:, 2 * k2 : 2 * k2 + 2, 512:B_LOC],
                                start=first,
                                stop=last,
                                perf_mode=DR,
                            )
                    if not evict:
                        return tiles
                    for gate in gates:
                        gt = j * 4 + gate
                        ps0, ps1 = tiles[gate]
                        g_sb = g_pool.tile([128, B_LOC], f32, tag="g")
                        func = AF.Tanh if gate == 2 else AF.Sigmoid
                        nc.scalar.activation(
                            g_sb[:, 0:512],
                            ps0[:],
                            func,
                            bias=bias_sb[:, gt : gt + 1],
                            scale=ISCALE,
                        )
                        nc.scalar.activation(
                            g_sb[:, 512:B_LOC],
                            ps1[:],
                            func,
                            bias=bias_sb[:, gt : gt + 1],
                            scale=ISCALE,
                        )
                        done[gate] = g_sb
                    return tiles

                done = {}
                if j == 0:
                    # single 4-way interleaved group: PE consumption matches
                    # the startup x-stream arrival rate
                    run_gates((0, 1, 2, 3))
                elif last_j:
                    for gates in ((2,), (0,), (1,)):
                        run_gates(gates)
                else:
                    for gates in ((0,), (1,), (2,), (3,)):
                        run_gates(gates)

                i_t, f_t, g_t = done[0], done[1], done[2]
                if not last_j:
                    o_t = done[3]
                    # c/h chain: runs on DVE/ACT while the next block's
                    # matmuls occupy the PE
                    c16 = out_pool.tile([128, B_LOC], bf16, tag="c")
                    tmp = out_pool.tile([128, B_LOC], f32, tag="tmp")
                    c0 = out_pool.tile([128, B_LOC], f32, tag="c0")
                    nc.vector.tensor_mul(out=tmp[:], in0=i_t[:], in1=g_t[:])
                    nc.vector.tensor_mul(out=c0[:], in0=f_t[:], in1=pc_sb[:])
                    nc.vector.tensor_add(out=c16[:], in0=c0[:], in1=tmp[:])
                    nc.scalar.dma_start(cT_d[j * 128 : (j + 1) * 128, :], c16[:])
                    h_sb = out_pool.tile([128, B_LOC], f32, tag="h")
                    nc.scalar.activation(h_sb[:], c16[:], AF.Tanh)
                    h16 = g_pool.tile([128, B_LOC], bf16, tag="h16")
                    nc.vector.tensor_mul(out=h16[:], in0=h_sb[:], in1=o_t[:])
                    nc.scalar.dma_start(hT_d[j * 128 : (j + 1) * 128, :], h16[:])
                else:
                    # final state block: o matmuls run while the c chain
                    # computes; the epilogue is pipelined in quarter chunks
                    # so the post-last-matmul serial chain is short
                    tiles = run_gates((3,), evict=False)
                    ps0, ps1 = tiles[3]
                    gt = j * 4 + 3
                    c16 = out_pool.tile([128, B_LOC], bf16, tag="c")
                    tmp = out_pool.tile([128, B_LOC], f32, tag="tmp")
                    c0 = out_pool.tile([128, B_LOC], f32, tag="c0")
                    h_sb = out_pool.tile([128, B_LOC], f32, tag="h")
                    o_sb = g_pool.tile([128, B_LOC], f32, tag="g")
                    h16 = g_pool.tile([128, B_LOC], bf16, tag="h16")
                    for cb in range(4):
                        ps = ps0 if cb < 2 else ps1
                        pslice = slice((cb % 2) * 256, (cb % 2) * 256 + 256)
                        qs = slice(cb * 256, (cb + 1) * 256)
                        nc.vector.tensor_mul(
                            out=tmp[:, qs], in0=i_t[:, qs], in1=g_t[:, qs]
                        )
                        nc.vector.tensor_mul(
                            out=c0[:, qs], in0=f_t[:, qs], in1=pc_sb[:, qs]
                        )
                        nc.vector.tensor_add(
                            out=c16[:, qs], in0=c0[:, qs], in1=tmp[:, qs]
                        )
                        nc.scalar.dma_start(
                            cT_d[j * 128 : (j + 1) * 128, qs], c16[:, qs]
                        )
                        nc.scalar.activation(h_sb[:, qs], c16[:, qs], AF.Tanh)
                        nc.scalar.activation(
                            o_sb[:, qs],
                            ps[:, pslice],
                            AF.Sigmoid,
                            bias=bias_sb[:, gt : gt + 1],
                            scale=ISCALE,
                        )
                        nc.vector.tensor_mul(
                            out=h16[:, qs], in0=o_sb[:, qs], in1=h_sb[:, qs]
                        )
                        nc.sync.dma_start(
                            hT_d[j * 128 : (j + 1) * 128, qs], h16[:, qs]
                        )

    nc.finalize()
    _install_wait_splitter(nc)
    return nc


def _split_multiwaits(mod: dict) -> dict:
    """This container's walrus encodes at most ONE sync wait per instruction
    (setupSyncWait raises 'Too many sync wait commands'), while Tile emits
    several. Move excess waits onto standalone single-wait EventSemaphore
    instructions inserted just before, on the same engine. All excess waits
    must be monotone (sem-ge-imm) for the serialization to be equivalent.
    """
    for fn in mod.get("functions", []):
        for blk in fn.get("blocks", []):
            insts = blk.get("instructions") or []
            out = []
            for inst in insts:
                si = inst.get("sync_info")
                waits = (si or {}).get("on_wait") or []
                if len(waits) > 1:
                    keep, extra = [], []
                    # keep non-monotone waits (if any) on the instruction
                    for w in waits:
                        (extra if w.get("wait_mode") == "sem-ge-imm" else keep).append(w)
                    if not keep:
                        keep.append(extra.pop())
                    for n, w in enumerate(extra):
                        out.append(
                            {
                                "name": f"{inst['name']}_sw{n}",
                                "opcode": "EventSemaphore",
                                "engine": inst["engine"],
                                "debug": inst.get("debug", 0),
                                "sync_info": {"on_wait": [w], "on_update": []},
                            }
                        )
                    si["on_wait"] = keep
                out.append(inst)
            blk["instructions"] = out
    return mod


def _install_wait_splitter(nc):
    import json as _json

    orig = nc.to_json_bytes

    def patched():
        mod = _json.loads(orig())
        return _json.dumps(_split_multiwaits(mod)).encode()

    nc.to_json_bytes = patched


def _quant(a, scale):
    """Scaled e4m3 value + same-scale residual (both as fp8)."""
    import ml_dtypes

    e4 = ml_dtypes.float8_e4m3
    s = np.clip(a * scale, -240.0, 240.0).astype(np.float32)
    q = s.astype(e4)
    r = np.clip(s - q.astype(np.float32), -240.0, 240.0).astype(e4)
    return q, r


def _prep_shared(Wx, bx, Wh):
    W = np.concatenate([Wx, Wh], axis=0)  # [K, 4*DIM]
    # columns gate*DIM + j*128 + c -> (j*4 + pos)*128 + c with device gate
    # order (i, f, g, o) within each state block j
    W_re = (
        W.reshape(K, 4, N_J, 128)[:, [0, 1, 3, 2]]
        .transpose(0, 2, 1, 3)
        .reshape(K, 4 * DIM)
    )
    W8, RW8 = _quant(W_re, SW)

    def dev_layout(A):  # [K, 4*DIM] fp8 -> [N_GT=(j,gate), 128, N_K2, 2, 128]
        return A.reshape(N_K2, 2, 128, N_GT, 128).transpose(3, 2, 0, 1, 4)

    w4 = dev_layout(W8).reshape(N_J, 4, 128, N_K2, 2, 128)
    rw1 = dev_layout(RW8).reshape(N_J, 4, 128, N_K2, 2, 128)[:, 2:3]
    # superblock: [j, 128, s(4 gates + rw), k2, 2, 128]
    w8_dev = np.ascontiguousarray(
        np.concatenate([w4, rw1], axis=1).transpose(0, 2, 1, 3, 4, 5)
    )
    b_re = bx.reshape(4, N_J, 128)[[0, 1, 3, 2]].transpose(1, 0, 2).reshape(4 * DIM)
    bias_dev = np.ascontiguousarray(b_re.reshape(N_GT, 128).T, dtype=np.float32)
    return w8_dev, bias_dev


def kernel(x, prevh, prevc, Wx, bx, Wh):
    import ml_dtypes
    from concourse import bass_utils

    bf16 = ml_dtypes.bfloat16
    x, prevh, prevc, Wx, bx, Wh = (
        np.asarray(a, dtype=np.float32) for a in (x, prevh, prevc, Wx, bx, Wh)
    )

    if "nc" not in _CACHED:
        _CACHED["nc"] = _build_program()
    nc = _CACHED["nc"]

    w8_dev, bias_dev = _prep_shared(Wx, bx, Wh)

    in_maps = []
    for c in range(NCORES):
        rows = slice(c * B_LOC, (c + 1) * B_LOC)
        xh = np.concatenate([x[rows], prevh[rows]], axis=1)  # [B_LOC, K]
        x8, rx8 = _quant(np.ascontiguousarray(xh.T), SX)  # [K, B_LOC]
        # pre-chunk to [partition, kt, col]
        x8 = np.ascontiguousarray(x8.reshape(N_KT, 128, B_LOC).transpose(1, 0, 2))
        rx8 = np.ascontiguousarray(rx8.reshape(N_KT, 128, B_LOC).transpose(1, 0, 2))
        pcT = np.ascontiguousarray(prevc[rows].T.astype(bf16))
        in_maps.append(
            {"x8": x8, "rx8": rx8, "w8": w8_dev, "bias": bias_dev, "pcT": pcT}
        )
    _CACHED["in_maps"] = in_maps

    res = bass_utils.run_bass_kernel_spmd(nc, in_maps, core_ids=list(range(NCORES)))

    nexth = np.empty((BATCH, DIM), np.float32)
    nextc = np.empty((BATCH, DIM), np.float32)
    for c in range(NCORES):
        rows = slice(c * B_LOC, (c + 1) * B_LOC)
        nexth[rows] = np.asarray(res.results[c]["hT"]).astype(np.float32).T
        nextc[rows] = np.asarray(res.results[c]["cT"]).astype(np.float32).T
    return nexth, nextc


if __name__ == "__main__":
    rng = np.random.default_rng(0)
    inputs = {
        "x": rng.standard_normal((BATCH, DIM), np.float32),
        "prevh": rng.standard_normal((BATCH, DIM), np.float32),
        "prevc": rng.standard_normal((BATCH, DIM), np.float32),
        "Wx": (rng.random((DIM, 4 * DIM), np.float32) - 0.5) / 16,
        "bx": (rng.random(4 * DIM, np.float32) - 0.5) / 16,
        "Wh": (rng.random((DIM, 4 * DIM), np.float32) - 0.5) / 16,
    }
    h, c = kernel(**inputs)
    print("ok", h.shape, c.shape, h.dtype)


# revision 9
# speedup vs baseline: 1.1326x; 1.0611x over previous
"""LSTM cell kernel for Trainium2, 8 NeuronCores, data-parallel over batch.

Math: stacked = x @ Wx + bx + prevh @ Wh
      i,f,o,g = split(stacked, 4, axis=1); i,f,o = sigmoid; g = tanh
      nextc = prevc*f + g*i ; nexth = tanh(nextc)*o

Device strategy (per core, batch shard of 1024 rows):
  - Host pre-concats [x|prevh] and [Wx;Wh] into one K=2048 contraction and
    quantizes both sides to scaled fp8 e4m3 (x*16, W*4096).  Matmuls run in
    MatmulPerfMode.DoubleRow: each instruction contracts 256 k-rows
    (2 x 128 partitions) at 0.5 cycles per output column - 4x the bf16
    row rate under the cost model.
  - Mixed per-gate precision keeps rel-err under the 2e-2 gate: i/f/o use a
    single fp8 pass; the tanh gate g (largest error sensitivity) accumulates
    three passes in PSUM: x8@W8 + rx8@W8 + x8@RW8, where rx8/RW8 are fp8
    quantization residuals at the same scale (effective ~bf16 accuracy).
  - DMA instruction count is minimized (the HWDGE device serializes at
    ~630ns/DMA): activations stream in 4 quarter-DMAs per tensor and each
    state block's 5 weight tiles (i,f,g,o,rw) arrive in one superblock DMA.
  - Per state block (device gate order i,f,g,o): evictions are emitted
    eviction-first so the in-order ACT queue frees PSUM banks promptly; the
    c/h elementwise chain runs after o's matmuls are issued and overlaps the
    next block on other engines.  j0 interleaves all four gates k-wise to
    match the startup x-stream rate; the last block runs g first and
    pipelines its epilogue in quarter chunks.
  - prevc loads and nexth/nextc stores are bf16 (negligible error, halves
    DMA traffic).  The 1/65536 fp8 scale is folded into the fused ACT
    eviction (func(psum*scale + bias)).  Outputs un-transposed on host.
"""

import os
import sys

sys.path.insert(0, "/opt/trn_rl_repo")
# v2 ASAP tile scheduler: measurably tighter schedule than the legacy flow
os.environ.setdefault("TILE_SCHEDULER", "asap")

import numpy as np

BATCH = 8192
DIM = 1024  # INPUT_DIM == STATE_DIM
K = 2 * DIM  # stacked contraction [x|prevh]
NCORES = 8
B_LOC = BATCH // NCORES  # 1024
N_KT = K // 128  # 16 k-tiles of 128
N_K2 = K // 256  # 8 DoubleRow k-steps of 256
N_GT = 4 * DIM // 128  # 32 gate-block tiles
N_J = DIM // 128  # 8 state blocks

SX = 16.0  # fp8 scale for activations
SW = 4096.0  # fp8 scale for weights
ISCALE = 1.0 / (SX * SW)

_CACHED = {}


def _build_program():
    import ml_dtypes  # noqa: F401
    from concourse import bass, tile
    from concourse.bass import mybir

    f8 = mybir.dt.float8e4
    bf16 = mybir.dt.bfloat16
    f32 = mybir.dt.float32
    AF = mybir.ActivationFunctionType
    DR = mybir.MatmulPerfMode.DoubleRow

    nc = bass.Bass("TRN2", target_bir_lowering=False)
    # activations pre-chunked on host: [partition, kt, col]
    x8_d = nc.dram_tensor("x8", [128, N_KT, B_LOC], f8, kind="ExternalInput")
    rx8_d = nc.dram_tensor("rx8", [128, N_KT, B_LOC], f8, kind="ExternalInput")
    # per-state-block weight superblock: s = 0..3 gates (i,f,g,o), 4 = rw(g)
    w8_d = nc.dram_tensor(
        "w8", [N_J, 128, 5, N_K2, 2, 128], f8, kind="ExternalInput"
    )
    bias_d = nc.dram_tensor("bias", [128, N_GT], f32, kind="ExternalInput")
    pcT_d = nc.dram_tensor("pcT", [DIM, B_LOC], bf16, kind="ExternalInput")
    hT_d = nc.dram_tensor("hT", [DIM, B_LOC], bf16, kind="ExternalOutput")
    cT_d = nc.dram_tensor("cT", [DIM, B_LOC], bf16, kind="ExternalOutput")

    with tile.TileContext(nc) as tc:
        with (
            tc.tile_pool(name="const", bufs=1) as const_pool,
            tc.tile_pool(name="wp", bufs=3) as w_pool,
            tc.tile_pool(name="pc", bufs=3) as pc_pool,
            tc.tile_pool(name="gates", bufs=10) as g_pool,
            tc.tile_pool(name="outs", bufs=4) as out_pool,
            tc.tile_pool(name="psum", bufs=8, space="PSUM") as psum_pool,
        ):
            # resident activations: x8 + residual, [128, kt, 1024] fp8,
            # 16KB/partition each.  A DoubleRow moving slice is
            # [:, 2*k2:2*k2+2, h*512:...] -> AP [128, 2, 512].
            xh8_sb = const_pool.tile([128, N_KT, B_LOC], f8)
            rx8_sb = const_pool.tile([128, N_KT, B_LOC], f8)
            bias_sb = const_pool.tile([128, N_GT], f32)

            # dummy matmuls first in PE program order: warm the PE HAM clock
            # gate (3us busy window) while the startup DMAs stream
            warm_sb = const_pool.tile([1, 128], f8)
            nc.gpsimd.memset(warm_sb[:], 0.0)
            warm_ps = psum_pool.tile([128, 512], f32, tag="ps")
            for _ in range(40):
                nc.tensor.matmul(
                    warm_ps[:, 0:64],
                    warm_sb[:, 0:128],
                    warm_sb[:, 0:64],
                    start=True,
                    stop=True,
                )

            w_tiles = {}

            def load_w(j):
                w_sb = w_pool.tile([128, 5, N_K2, 2, 128], f8, tag="w")
                nc.sync.dma_start(w_sb[:], w8_d[j])
                w_tiles[j] = w_sb

            # startup: j0 weights and the x stream on sync (split small for a
            # fast PE start), residuals and bias on the scalar hwdge queue
            w0_sb = w_pool.tile([128, 5, N_K2, 2, 128], f8, tag="w")
            w_tiles[0] = w0_sb
            nc.sync.dma_start(w0_sb[:, 0:1], w8_d[0][:, 0:1])
            nc.sync.dma_start(xh8_sb[:, 0:2], x8_d[:, 0:2])
            nc.sync.dma_start(w0_sb[:, 1:2], w8_d[0][:, 1:2])
            nc.sync.dma_start(xh8_sb[:, 2:4], x8_d[:, 2:4])
            nc.scalar.dma_start(bias_sb[:], bias_d[:])
            nc.sync.dma_start(w0_sb[:, 2:5], w8_d[0][:, 2:5])
            nc.sync.dma_start(xh8_sb[:, 4:8], x8_d[:, 4:8])
            nc.sync.dma_start(xh8_sb[:, 8:12], x8_d[:, 8:12])
            load_w(1)  # before the rx stream: j1 must start right after j0
            nc.sync.dma_start(xh8_sb[:, 12:16], x8_d[:, 12:16])
            for q in range(4):
                nc.scalar.dma_start(
                    rx8_sb[:, 4 * q : 4 * q + 4], rx8_d[:, 4 * q : 4 * q + 4]
                )

            for j in range(N_J):
                last_j = j == N_J - 1
                # prefetch next block's weight superblock; load this block's
                # prevc early (no deps, so it never parks the ACT queue)
                if j >= 1 and not last_j:
                    load_w(j + 1)
                pc_sb = pc_pool.tile([128, B_LOC], bf16)
                nc.sync.dma_start(pc_sb[:], pcT_d[j * 128 : (j + 1) * 128, :])
                w_all = w_tiles.pop(j)

                def gate_passes(gate):
                    """(stationary s-index, moving tile) per accumulation
                    pass."""
                    if gate == 2:
                        if j == 0:  # rx arrives last in the startup stream
                            return [(2, xh8_sb), (4, xh8_sb), (2, rx8_sb)]
                        return [(2, xh8_sb), (2, rx8_sb), (4, xh8_sb)]
                    return [(gate, xh8_sb)]

                def run_gates(gates, evict=True):
                    """Issue DoubleRow matmuls for one or more gates,
                    k-interleaved when len>1, then evict eviction-first so
                    the in-order ACT queue frees PSUM banks promptly."""
                    tiles = {}
                    steps = {}
                    for gate in gates:
                        ps0 = psum_pool.tile([128, 512], f32, tag="ps")
                        ps1 = psum_pool.tile([128, 512], f32, tag="ps")
                        tiles[gate] = (ps0, ps1)
                        vk = []
                        for s, m_sb in gate_passes(gate):
                            for k2 in range(N_K2):
                                vk.append((s, m_sb, k2))
                        steps[gate] = vk
                    n_steps = max(len(s) for s in steps.values())
                    for vi in range(n_steps):
                        for gate in gates:
                            vk = steps[gate]
                            if vi >= len(vk):
                                continue
                            s, m_sb, k2 = vk[vi]
                            first, last = vi == 0, vi == len(vk) - 1
                            ps0, ps1 = tiles[gate]
                            lhsT = w_all[:, s, k2]
                            nc.tensor.matmul(
                                ps0[:],
                                lhsT,
                                m_sb[:, 2 * k2 : 2 * k2 + 2, 0:512],
                                start=first,
                                stop=last,
                                perf_mode=DR,
                            )
                            nc.tensor.matmul(
                                ps1[:],
                                lhsT,
                                m_sb[:, 2 * k2 : 2 * k2 + 2, 512:B_LOC],
                                start=first,
                                stop=last,
                                perf_mode=DR,
                            )
                    if not evict:
                        return tiles
                    for gate in gates:
                        gt = j * 4 + gate
                        ps0, ps1 = tiles[gate]
                        g_sb = g_pool.tile([128, B_LOC], f32, tag="g")
                        func = AF.Tanh if gate == 2 else AF.Sigmoid
                        nc.scalar.activation(
                            g_sb[:, 0:512],
                            ps0[:],
                            func,
                            bias=bias_sb[:, gt : gt + 1],
                            scale=ISCALE,
                        )
                        nc.scalar.activation(
                            g_sb[:, 512:B_LOC],
                            ps1[:],
                            func,
                            bias=bias_sb[:, gt : gt + 1],
                            scale=ISCALE,
                        )
                        done[gate] = g_sb
                    return tiles

                done = {}
                if j == 0:
                    # single 4-way interleaved group: PE consumption matches
                    # the startup x-stream arrival rate
                    run_gates((0, 1, 2, 3))
                elif last_j:
                    for gates in ((2,), (0,), (1,)):
                        run_gates(gates)
                else:
                    for gates in ((0,), (1,), (2,), (3,)):
                        run_gates(gates)

                i_t, f_t, g_t = done[0], done[1], done[2]
                if not last_j:
                    o_t = done[3]
                    # c/h chain: runs on DVE/ACT while the next block's
                    # matmuls occupy the PE
                    c16 = out_pool.tile([128, B_LOC], bf16, tag="c")
                    tmp = out_pool.tile([128, B_LOC], f32, tag="tmp")
                    c0 = out_pool.tile([128, B_LOC], f32, tag="c0")
                    nc.vector.tensor_mul(out=tmp[:], in0=i_t[:], in1=g_t[:])
                    nc.vector.tensor_mul(out=c0[:], in0=f_t[:], in1=pc_sb[:])
                    nc.vector.tensor_add(out=c16[:], in0=c0[:], in1=tmp[:])
                    nc.sync.dma_start(cT_d[j * 128 : (j + 1) * 128, :], c16[:])
                    h_sb = out_pool.tile([128, B_LOC], f32, tag="h")
                    nc.scalar.activation(h_sb[:], c16[:], AF.Tanh)
                    h16 = g_pool.tile([128, B_LOC], bf16, tag="h16")
                    nc.vector.tensor_mul(out=h16[:], in0=h_sb[:], in1=o_t[:])
                    nc.sync.dma_start(hT_d[j * 128 : (j + 1) * 128, :], h16[:])
                else:
                    # final state block: o matmuls run while the c chain
                    # computes; the epilogue is pipelined in quarter chunks
                    # so the post-last-matmul serial chain is short
                    tiles = run_gates((3,), evict=False)
                    ps0, ps1 = tiles[3]
                    gt = j * 4 + 3
                    c16 = out_pool.tile([128, B_LOC], bf16, tag="c")
                    tmp = out_pool.tile([128, B_LOC], f32, tag="tmp")
                    c0 = out_pool.tile([128, B_LOC], f32, tag="c0")
                    h_sb = out_pool.tile([128, B_LOC], f32, tag="h")
                    o_sb = g_pool.tile([128, B_LOC], f32, tag="g")
                    h16 = g_pool.tile([128, B_LOC], bf16, tag="h16")
                    for cb in range(4):
                        ps = ps0 if cb < 2 else ps1
                        pslice = slice((cb % 2) * 256, (cb % 2) * 256 + 256)
                        qs = slice(cb * 256, (cb + 1) * 256)
                        nc.vector.tensor_mul(
                            out=tmp[:, qs], in0=i_t[:, qs], in1=g_t[:, qs]
                        )
                        nc.vector.tensor_mul(
                            out=c0[:, qs], in0=f_t[:, qs], in1=pc_sb[:, qs]
                        )
                        nc.vector.tensor_add(
                            out=c16[:, qs], in0=c0[:, qs], in1=tmp[:, qs]
                        )
                        nc.sync.dma_start(
                            cT_d[j * 128 : (j + 1) * 128, qs], c16[:, qs]
                        )
                        nc.scalar.activation(h_sb[:, qs], c16[:, qs], AF.Tanh)
                        nc.scalar.activation(
                            o_sb[:, qs],
                            ps[:, pslice],
                            AF.Sigmoid,
                            bias=bias_sb[:, gt : gt + 1],
                            scale=ISCALE,
                        )
                        nc.vector.tensor_mul(
                            out=h16[:, qs], in0=o_sb[:, qs], in1=h_sb[:, qs]
                        )
                        nc.sync.dma_start(
                            hT_d[j * 128 : (j + 1) * 128, qs], h16[:, qs]
                        )

    nc.finalize()
    _install_wait_splitter(nc)
    return nc


def _split_multiwaits(mod: dict) -> dict:
    """This container's walrus encodes at most ONE sync wait per instruction
    (setupSyncWait raises 'Too many sync wait commands'), while Tile emits
    several. Move excess waits onto standalone single-wait EventSemaphore
    instructions inserted just before, on the same engine. All excess waits
    must be monotone (sem-ge-imm) for the serialization to be equivalent.
    """
    for fn in mod.get("functions", []):
        for blk in fn.get("blocks", []):
            insts = blk.get("instructions") or []
            out = []
            for inst in insts:
                si = inst.get("sync_info")
                waits = (si or {}).get("on_wait") or []
                if len(waits) > 1:
                    keep, extra = [], []
                    # keep non-monotone waits (if any) on the instruction
                    for w in waits:
                        (extra if w.get("wait_mode") == "sem-ge-imm" else keep).append(w)
                    if not keep:
                        keep.append(extra.pop())
                    for n, w in enumerate(extra):
                        out.append(
                            {
                                "name": f"{inst['name']}_sw{n}",
                                "opcode": "EventSemaphore",
                                "engine": inst["engine"],
                                "debug": inst.get("debug", 0),
                                "sync_info": {"on_wait": [w], "on_update": []},
                            }
                        )
                    si["on_wait"] = keep
                out.append(inst)
            blk["instructions"] = out
    return mod


def _install_wait_splitter(nc):
    import json as _json

    orig = nc.to_json_bytes

    def patched():
        mod = _json.loads(orig())
        return _json.dumps(_split_multiwaits(mod)).encode()

    nc.to_json_bytes = patched


def _quant(a, scale):
    """Scaled e4m3 value + same-scale residual (both as fp8)."""
    import ml_dtypes

    e4 = ml_dtypes.float8_e4m3
    s = np.clip(a * scale, -240.0, 240.0).astype(np.float32)
    q = s.astype(e4)
    r = np.clip(s - q.astype(np.float32), -240.0, 240.0).astype(e4)
    return q, r


def _prep_shared(Wx, bx, Wh):
    W = np.concatenate([Wx, Wh], axis=0)  # [K, 4*DIM]
    # columns gate*DIM + j*128 + c -> (j*4 + pos)*128 + c with device gate
    # order (i, f, g, o) within each state block j
    W_re = (
        W.reshape(K, 4, N_J, 128)[:, [0, 1, 3, 2]]
        .transpose(0, 2, 1, 3)
        .reshape(K, 4 * DIM)
    )
    W8, RW8 = _quant(W_re, SW)

    def dev_layout(A):  # [K, 4*DIM] fp8 -> [N_GT=(j,gate), 128, N_K2, 2, 128]
        return A.reshape(N_K2, 2, 128, N_GT, 128).transpose(3, 2, 0, 1, 4)

    w4 = dev_layout(W8).reshape(N_J, 4, 128, N_K2, 2, 128)
    rw1 = dev_layout(RW8).reshape(N_J, 4, 128, N_K2, 2, 128)[:, 2:3]
    # superblock: [j, 128, s(4 gates + rw), k2, 2, 128]
    w8_dev = np.ascontiguousarray(
        np.concatenate([w4, rw1], axis=1).transpose(0, 2, 1, 3, 4, 5)
    )
    b_re = bx.reshape(4, N_J, 128)[[0, 1, 3, 2]].transpose(1, 0, 2).reshape(4 * DIM)
    bias_dev = np.ascontiguousarray(b_re.reshape(N_GT, 128).T, dtype=np.float32)
    return w8_dev, bias_dev


def kernel(x, prevh, prevc, Wx, bx, Wh):
    import ml_dtypes
    from concourse import bass_utils

    bf16 = ml_dtypes.bfloat16
    x, prevh, prevc, Wx, bx, Wh = (
        np.asarray(a, dtype=np.float32) for a in (x, prevh, prevc, Wx, bx, Wh)
    )

    if "nc" not in _CACHED:
        _CACHED["nc"] = _build_program()
    nc = _CACHED["nc"]

    w8_dev, bias_dev = _prep_shared(Wx, bx, Wh)

    in_maps = []
    for c in range(NCORES):
        rows = slice(c * B_LOC, (c + 1) * B_LOC)
        xh = np.concatenate([x[rows], prevh[rows]], axis=1)  # [B_LOC, K]
        x8, rx8 = _quant(np.ascontiguousarray(xh.T), SX)  # [K, B_LOC]
        # pre-chunk to [partition, kt, col]
        x8 = np.ascontiguousarray(x8.reshape(N_KT, 128, B_LOC).transpose(1, 0, 2))
        rx8 = np.ascontiguousarray(rx8.reshape(N_KT, 128, B_LOC).transpose(1, 0, 2))
        pcT = np.ascontiguousarray(prevc[rows].T.astype(bf16))
        in_maps.append(
            {"x8": x8, "rx8": rx8, "w8": w8_dev, "bias": bias_dev, "pcT": pcT}
        )
    _CACHED["in_maps"] = in_maps

    res = bass_utils.run_bass_kernel_spmd(nc, in_maps, core_ids=list(range(NCORES)))

    nexth = np.empty((BATCH, DIM), np.float32)
    nextc = np.empty((BATCH, DIM), np.float32)
    for c in range(NCORES):
        rows = slice(c * B_LOC, (c + 1) * B_LOC)
        nexth[rows] = np.asarray(res.results[c]["hT"]).astype(np.float32).T
        nextc[rows] = np.asarray(res.results[c]["cT"]).astype(np.float32).T
    return nexth, nextc


if __name__ == "__main__":
    rng = np.random.default_rng(0)
    inputs = {
        "x": rng.standard_normal((BATCH, DIM), np.float32),
        "prevh": rng.standard_normal((BATCH, DIM), np.float32),
        "prevc": rng.standard_normal((BATCH, DIM), np.float32),
        "Wx": (rng.random((DIM, 4 * DIM), np.float32) - 0.5) / 16,
        "bx": (rng.random(4 * DIM, np.float32) - 0.5) / 16,
        "Wh": (rng.random((DIM, 4 * DIM), np.float32) - 0.5) / 16,
    }
    h, c = kernel(**inputs)
    print("ok", h.shape, c.shape, h.dtype)


# revision 11
# speedup vs baseline: 1.3585x; 1.1995x over previous
"""LSTM cell kernel for Trainium2, 8 NeuronCores, data-parallel over batch.

Math: stacked = x @ Wx + bx + prevh @ Wh
      i,f,o,g = split(stacked, 4, axis=1); i,f,o = sigmoid; g = tanh
      nextc = prevc*f + g*i ; nexth = tanh(nextc)*o

Device strategy (per core, batch shard of 1024 rows):
  - Host pre-concats [x|prevh] and [Wx;Wh] into one K=2048 contraction and
    quantizes both sides to scaled fp8 e4m3 (x*16, W*4096).  Matmuls run in
    MatmulPerfMode.DoubleRow: each instruction contracts 256 k-rows
    (2 x 128 partitions) at 0.5 cycles per output column - 4x the bf16
    row rate under the cost model.
  - Mixed per-gate precision keeps rel-err under the 2e-2 gate: i/f/o use a
    single fp8 pass; the tanh gate g (largest error sensitivity) accumulates
    three passes in PSUM: x8@W8 + rx8@W8 + x8@RW8, where rx8/RW8 are fp8
    quantization residuals at the same scale (effective ~bf16 accuracy).
  - DMA instruction count is minimized (the HWDGE device serializes at
    ~630ns/DMA): activations stream in 4 quarter-DMAs per tensor and each
    state block's 5 weight tiles (i,f,g,o,rw) arrive in one superblock DMA.
  - Per state block (device gate order i,f,g,o): evictions are emitted
    eviction-first so the in-order ACT queue frees PSUM banks promptly; the
    c/h elementwise chain runs after o's matmuls are issued and overlaps the
    next block on other engines.  j0 interleaves all four gates k-wise to
    match the startup x-stream rate; the last block runs g first and
    pipelines its epilogue in quarter chunks.
  - prevc loads and nexth/nextc stores are bf16 (negligible error, halves
    DMA traffic).  The 1/65536 fp8 scale is folded into the fused ACT
    eviction (func(psum*scale + bias)).  Outputs un-transposed on host.
"""

import os
import sys

sys.path.insert(0, "/opt/trn_rl_repo")
# v2 ASAP tile scheduler: measurably tighter schedule than the legacy flow
os.environ.setdefault("TILE_SCHEDULER", "asap")

import numpy as np

BATCH = 8192
DIM = 1024  # INPUT_DIM == STATE_DIM
K = 2 * DIM  # stacked contraction [x|prevh]
NCORES = 8
B_LOC = BATCH // NCORES  # 1024
N_KT = K // 128  # 16 k-tiles of 128
N_K2 = K // 256  # 8 DoubleRow k-steps of 256
N_GT = 4 * DIM // 128  # 32 gate-block tiles
N_J = DIM // 128  # 8 state blocks

SX = 16.0  # fp8 scale for activations
SW = 4096.0  # fp8 scale for weights
ISCALE = 1.0 / (SX * SW)

_CACHED = {}


def _build_program():
    import ml_dtypes  # noqa: F401
    from concourse import bass, tile
    from concourse.bass import mybir

    f8 = mybir.dt.float8e4
    bf16 = mybir.dt.bfloat16
    f32 = mybir.dt.float32
    AF = mybir.ActivationFunctionType
    DR = mybir.MatmulPerfMode.DoubleRow

    nc = bass.Bass("TRN2", target_bir_lowering=False)
    # activations pre-chunked on host: [partition, kt, col]
    x8_d = nc.dram_tensor("x8", [128, N_KT, B_LOC], f8, kind="ExternalInput")
    rx8_d = nc.dram_tensor("rx8", [128, N_KT, B_LOC], f8, kind="ExternalInput")
    # per-state-block weight superblock: s = 0..3 gates (i,f,g,o), 4 = rw(g)
    w8_d = nc.dram_tensor(
        "w8", [N_J, 128, 5, N_K2, 2, 128], f8, kind="ExternalInput"
    )
    bias_d = nc.dram_tensor("bias", [128, N_GT], f32, kind="ExternalInput")
    pcT_d = nc.dram_tensor("pcT", [DIM, B_LOC], bf16, kind="ExternalInput")
    hT_d = nc.dram_tensor("hT", [DIM, B_LOC], bf16, kind="ExternalOutput")
    cT_d = nc.dram_tensor("cT", [DIM, B_LOC], bf16, kind="ExternalOutput")

    with tile.TileContext(nc) as tc:
        with (
            tc.tile_pool(name="const", bufs=1) as const_pool,
            tc.tile_pool(name="wp", bufs=3) as w_pool,
            tc.tile_pool(name="pc", bufs=3) as pc_pool,
            tc.tile_pool(name="gates", bufs=10) as g_pool,
            tc.tile_pool(name="outs", bufs=4) as out_pool,
            tc.tile_pool(name="psum", bufs=8, space="PSUM") as psum_pool,
        ):
            # resident activations: x8 + residual, [128, kt, 1024] fp8,
            # 16KB/partition each.  A DoubleRow moving slice is
            # [:, 2*k2:2*k2+2, h*512:...] -> AP [128, 2, 512].
            xh8_sb = const_pool.tile([128, N_KT, B_LOC], f8)
            rx8_sb = const_pool.tile([128, N_KT, B_LOC], f8)
            bias_sb = const_pool.tile([128, N_GT], f32)

            # dummy matmuls first in PE program order: warm the PE HAM clock
            # gate (3us busy window) while the startup DMAs stream
            warm_sb = const_pool.tile([1, 128], f8)
            nc.gpsimd.memset(warm_sb[:], 0.0)
            warm_ps = psum_pool.tile([128, 512], f32, tag="ps")
            for _ in range(40):
                nc.tensor.matmul(
                    warm_ps[:, 0:64],
                    warm_sb[:, 0:128],
                    warm_sb[:, 0:64],
                    start=True,
                    stop=True,
                )

            w_tiles = {}

            def load_w(j):
                # blocks j>=1 need only the 4 gate tiles (their g residual
                # pass uses rx, not rw)
                w_sb = w_pool.tile([128, 4, N_K2, 2, 128], f8, tag="w")
                nc.sync.dma_start(w_sb[:], w8_d[j][:, 0:4])
                w_tiles[j] = w_sb

            # startup: j0 weights and the x stream on sync (split small for a
            # fast PE start), residuals and bias on the scalar hwdge queue
            w0_sb = const_pool.tile([128, 5, N_K2, 2, 128], f8)
            w_tiles[0] = w0_sb
            nc.sync.dma_start(w0_sb[:, 0:1], w8_d[0][:, 0:1])
            nc.sync.dma_start(xh8_sb[:, 0:2], x8_d[:, 0:2])
            nc.sync.dma_start(w0_sb[:, 1:2], w8_d[0][:, 1:2])
            nc.sync.dma_start(xh8_sb[:, 2:4], x8_d[:, 2:4])
            nc.scalar.dma_start(bias_sb[:], bias_d[:])
            nc.sync.dma_start(w0_sb[:, 2:5], w8_d[0][:, 2:5])
            nc.sync.dma_start(xh8_sb[:, 4:8], x8_d[:, 4:8])
            nc.sync.dma_start(xh8_sb[:, 8:12], x8_d[:, 8:12])
            nc.sync.dma_start(xh8_sb[:, 12:16], x8_d[:, 12:16])
            load_w(1)
            for q in range(4):
                nc.scalar.dma_start(
                    rx8_sb[:, 4 * q : 4 * q + 4], rx8_d[:, 4 * q : 4 * q + 4]
                )

            for j in range(N_J):
                last_j = j == N_J - 1
                # prefetch next block's weight superblock; load this block's
                # prevc early (no deps, so it never parks the ACT queue)
                if j >= 1 and not last_j:
                    load_w(j + 1)
                pc_sb = pc_pool.tile([128, B_LOC], bf16)
                nc.sync.dma_start(pc_sb[:], pcT_d[j * 128 : (j + 1) * 128, :])
                w_all = w_tiles.pop(j)

                def gate_passes(gate):
                    """(stationary s-index, moving tile) per accumulation
                    pass: the g gate adds one same-scale residual pass (the
                    weight residual for j0 - rx arrives late in the startup
                    stream - and the activation residual elsewhere)."""
                    if gate == 2:
                        if j == 0:
                            return [(2, xh8_sb), (4, xh8_sb)]
                        return [(2, xh8_sb), (2, rx8_sb)]
                    return [(gate, xh8_sb)]

                def run_gates(gates, evict=True, half_major=False):
                    """Issue DoubleRow matmuls for one or more gates,
                    k-interleaved when len>1, then evict eviction-first so
                    the in-order ACT queue frees PSUM banks promptly.  With
                    half_major, a single gate emits all batch-half-0 matmuls
                    before half 1 so ps0 stops (and can evict) early."""
                    tiles = {}
                    steps = {}
                    for gate in gates:
                        ps0 = psum_pool.tile([128, 512], f32, tag="ps")
                        ps1 = psum_pool.tile([128, 512], f32, tag="ps")
                        tiles[gate] = (ps0, ps1)
                        vk = []
                        for s, m_sb in gate_passes(gate):
                            for k2 in range(N_K2):
                                vk.append((s, m_sb, k2))
                        steps[gate] = vk
                    n_steps = max(len(s) for s in steps.values())
                    if half_major:
                        (gate,) = gates
                        vk = steps[gate]
                        ps0, ps1 = tiles[gate]
                        for ps, cols in ((ps0, slice(0, 512)), (ps1, slice(512, B_LOC))):
                            for vi, (s, m_sb, k2) in enumerate(vk):
                                nc.tensor.matmul(
                                    ps[:],
                                    w_all[:, s, k2],
                                    m_sb[:, 2 * k2 : 2 * k2 + 2, cols],
                                    start=vi == 0,
                                    stop=vi == len(vk) - 1,
                                    perf_mode=DR,
                                )
                    else:
                        for vi in range(n_steps):
                            for gate in gates:
                                vk = steps[gate]
                                if vi >= len(vk):
                                    continue
                                s, m_sb, k2 = vk[vi]
                                first, last = vi == 0, vi == len(vk) - 1
                                ps0, ps1 = tiles[gate]
                                lhsT = w_all[:, s, k2]
                                nc.tensor.matmul(
                                    ps0[:],
                                    lhsT,
                                    m_sb[:, 2 * k2 : 2 * k2 + 2, 0:512],
                                    start=first,
                                    stop=last,
                                    perf_mode=DR,
                                )
                                nc.tensor.matmul(
                                    ps1[:],
                                    lhsT,
                                    m_sb[:, 2 * k2 : 2 * k2 + 2, 512:B_LOC],
                                    start=first,
                                    stop=last,
                                    perf_mode=DR,
                                )
                    if not evict:
                        return tiles
                    for gate in gates:
                        gt = j * 4 + gate
                        ps0, ps1 = tiles[gate]
                        g_sb = g_pool.tile([128, B_LOC], f32, tag="g")
                        func = AF.Tanh if gate == 2 else AF.Sigmoid
                        nc.scalar.activation(
                            g_sb[:, 0:512],
                            ps0[:],
                            func,
                            bias=bias_sb[:, gt : gt + 1],
                            scale=ISCALE,
                        )
                        nc.scalar.activation(
                            g_sb[:, 512:B_LOC],
                            ps1[:],
                            func,
                            bias=bias_sb[:, gt : gt + 1],
                            scale=ISCALE,
                        )
                        done[gate] = g_sb
                    return tiles

                done = {}
                if j == 0:
                    # single 4-way interleaved group: PE consumption matches
                    # the startup x-stream arrival rate
                    run_gates((0, 1, 2, 3))
                elif last_j:
                    for gates in ((2,), (0,), (1,)):
                        run_gates(gates)
                else:
                    for gates in ((0,), (1,), (2,), (3,)):
                        run_gates(gates)

                i_t, f_t, g_t = done[0], done[1], done[2]
                if not last_j:
                    o_t = done[3]
                    # c/h chain: runs on DVE/ACT while the next block's
                    # matmuls occupy the PE
                    c16 = out_pool.tile([128, B_LOC], bf16, tag="c")
                    tmp = out_pool.tile([128, B_LOC], f32, tag="tmp")
                    c0 = out_pool.tile([128, B_LOC], f32, tag="c0")
                    nc.vector.tensor_mul(out=tmp[:], in0=i_t[:], in1=g_t[:])
                    nc.vector.tensor_mul(out=c0[:], in0=f_t[:], in1=pc_sb[:])
                    nc.vector.tensor_add(out=c16[:], in0=c0[:], in1=tmp[:])
                    nc.sync.dma_start(cT_d[j * 128 : (j + 1) * 128, :], c16[:])
                    h_sb = out_pool.tile([128, B_LOC], f32, tag="h")
                    nc.scalar.activation(h_sb[:], c16[:], AF.Tanh)
                    h16 = g_pool.tile([128, B_LOC], bf16, tag="h16")
                    nc.vector.tensor_mul(out=h16[:], in0=h_sb[:], in1=o_t[:])
                    nc.sync.dma_start(hT_d[j * 128 : (j + 1) * 128, :], h16[:])
                else:
                    # final state block: o matmuls run half-major while the c
                    # chain computes; the epilogue pipelines in batch halves
                    # so the post-last-matmul serial chain is short
                    tiles = run_gates((3,), evict=False, half_major=True)
                    ps0, ps1 = tiles[3]
                    gt = j * 4 + 3
                    c16 = out_pool.tile([128, B_LOC], bf16, tag="c")
                    tmp = out_pool.tile([128, B_LOC], f32, tag="tmp")
                    c0 = out_pool.tile([128, B_LOC], f32, tag="c0")
                    h_sb = out_pool.tile([128, B_LOC], f32, tag="h")
                    o_sb = g_pool.tile([128, B_LOC], f32, tag="g")
                    h16 = g_pool.tile([128, B_LOC], bf16, tag="h16")
                    for hb, ps in ((0, ps0), (1, ps1)):
                        hs = slice(hb * 512, (hb + 1) * 512)
                        nc.vector.tensor_mul(
                            out=tmp[:, hs], in0=i_t[:, hs], in1=g_t[:, hs]
                        )
                        nc.vector.tensor_mul(
                            out=c0[:, hs], in0=f_t[:, hs], in1=pc_sb[:, hs]
                        )
                        nc.vector.tensor_add(
                            out=c16[:, hs], in0=c0[:, hs], in1=tmp[:, hs]
                        )
                        nc.sync.dma_start(
                            cT_d[j * 128 : (j + 1) * 128, hs], c16[:, hs]
                        )
                        nc.scalar.activation(
                            o_sb[:, hs],
                            ps[:],
                            AF.Sigmoid,
                            bias=bias_sb[:, gt : gt + 1],
                            scale=ISCALE,
                        )
                        nc.scalar.activation(h_sb[:, hs], c16[:, hs], AF.Tanh)
                        nc.vector.tensor_mul(
                            out=h16[:, hs], in0=o_sb[:, hs], in1=h_sb[:, hs]
                        )
                        nc.sync.dma_start(
                            hT_d[j * 128 : (j + 1) * 128, hs], h16[:, hs]
                        )

    nc.finalize()
    _install_wait_splitter(nc)
    return nc


def _split_multiwaits(mod: dict) -> dict:
    """This container's walrus encodes at most ONE sync wait per instruction
    (setupSyncWait raises 'Too many sync wait commands'), while Tile emits
    several. Move excess waits onto standalone single-wait EventSemaphore
    instructions inserted just before, on the same engine. All excess waits
    must be monotone (sem-ge-imm) for the serialization to be equivalent.
    """
    for fn in mod.get("functions", []):
        for blk in fn.get("blocks", []):
            insts = blk.get("instructions") or []
            out = []
            for inst in insts:
                si = inst.get("sync_info")
                waits = (si or {}).get("on_wait") or []
                if len(waits) > 1:
                    keep, extra = [], []
                    # keep non-monotone waits (if any) on the instruction
                    for w in waits:
                        (extra if w.get("wait_mode") == "sem-ge-imm" else keep).append(w)
                    if not keep:
                        keep.append(extra.pop())
                    for n, w in enumerate(extra):
                        out.append(
                            {
                                "name": f"{inst['name']}_sw{n}",
                                "opcode": "EventSemaphore",
                                "engine": inst["engine"],
                                "debug": inst.get("debug", 0),
                                "sync_info": {"on_wait": [w], "on_update": []},
                            }
                        )
                    si["on_wait"] = keep
                out.append(inst)
            blk["instructions"] = out
    return mod


def _install_wait_splitter(nc):
    import json as _json

    orig = nc.to_json_bytes

    def patched():
        mod = _json.loads(orig())
        return _json.dumps(_split_multiwaits(mod)).encode()

    nc.to_json_bytes = patched


def _quant(a, scale):
    """Scaled e4m3 value + same-scale residual (both as fp8)."""
    import ml_dtypes

    e4 = ml_dtypes.float8_e4m3
    s = np.clip(a * scale, -240.0, 240.0).astype(np.float32)
    q = s.astype(e4)
    r = np.clip(s - q.astype(np.float32), -240.0, 240.0).astype(e4)
    return q, r


def _prep_shared(Wx, bx, Wh):
    W = np.concatenate([Wx, Wh], axis=0)  # [K, 4*DIM]
    # columns gate*DIM + j*128 + c -> (j*4 + pos)*128 + c with device gate
    # order (i, f, g, o) within each state block j
    W_re = (
        W.reshape(K, 4, N_J, 128)[:, [0, 1, 3, 2]]
        .transpose(0, 2, 1, 3)
        .reshape(K, 4 * DIM)
    )
    W8, RW8 = _quant(W_re, SW)

    def dev_layout(A):  # [K, 4*DIM] fp8 -> [N_GT=(j,gate), 128, N_K2, 2, 128]
        return A.reshape(N_K2, 2, 128, N_GT, 128).transpose(3, 2, 0, 1, 4)

    w4 = dev_layout(W8).reshape(N_J, 4, 128, N_K2, 2, 128)
    rw1 = dev_layout(RW8).reshape(N_J, 4, 128, N_K2, 2, 128)[:, 2:3]
    # superblock: [j, 128, s(4 gates + rw), k2, 2, 128]
    w8_dev = np.ascontiguousarray(
        np.concatenate([w4, rw1], axis=1).transpose(0, 2, 1, 3, 4, 5)
    )
    b_re = bx.reshape(4, N_J, 128)[[0, 1, 3, 2]].transpose(1, 0, 2).reshape(4 * DIM)
    bias_dev = np.ascontiguousarray(b_re.reshape(N_GT, 128).T, dtype=np.float32)
    return w8_dev, bias_dev


def kernel(x, prevh, prevc, Wx, bx, Wh):
    import ml_dtypes
    from concourse import bass_utils

    bf16 = ml_dtypes.bfloat16
    x, prevh, prevc, Wx, bx, Wh = (
        np.asarray(a, dtype=np.float32) for a in (x, prevh, prevc, Wx, bx, Wh)
    )

    if "nc" not in _CACHED:
        _CACHED["nc"] = _build_program()
    nc = _CACHED["nc"]

    w8_dev, bias_dev = _prep_shared(Wx, bx, Wh)

    in_maps = []
    for c in range(NCORES):
        rows = slice(c * B_LOC, (c + 1) * B_LOC)
        xh = np.concatenate([x[rows], prevh[rows]], axis=1)  # [B_LOC, K]
        x8, rx8 = _quant(np.ascontiguousarray(xh.T), SX)  # [K, B_LOC]
        # pre-chunk to [partition, kt, col]
        x8 = np.ascontiguousarray(x8.reshape(N_KT, 128, B_LOC).transpose(1, 0, 2))
        rx8 = np.ascontiguousarray(rx8.reshape(N_KT, 128, B_LOC).transpose(1, 0, 2))
        pcT = np.ascontiguousarray(prevc[rows].T.astype(bf16))
        in_maps.append(
            {"x8": x8, "rx8": rx8, "w8": w8_dev, "bias": bias_dev, "pcT": pcT}
        )
    _CACHED["in_maps"] = in_maps

    res = bass_utils.run_bass_kernel_spmd(nc, in_maps, core_ids=list(range(NCORES)))

    nexth = np.empty((BATCH, DIM), np.float32)
    nextc = np.empty((BATCH, DIM), np.float32)
    for c in range(NCORES):
        rows = slice(c * B_LOC, (c + 1) * B_LOC)
        nexth[rows] = np.asarray(res.results[c]["hT"]).astype(np.float32).T
        nextc[rows] = np.asarray(res.results[c]["cT"]).astype(np.float32).T
    return nexth, nextc


if __name__ == "__main__":
    rng = np.random.default_rng(0)
    inputs = {
        "x": rng.standard_normal((BATCH, DIM), np.float32),
        "prevh": rng.standard_normal((BATCH, DIM), np.float32),
        "prevc": rng.standard_normal((BATCH, DIM), np.float32),
        "Wx": (rng.random((DIM, 4 * DIM), np.float32) - 0.5) / 16,
        "bx": (rng.random(4 * DIM, np.float32) - 0.5) / 16,
        "Wh": (rng.random((DIM, 4 * DIM), np.float32) - 0.5) / 16,
    }
    h, c = kernel(**inputs)
    print("ok", h.shape, c.shape, h.dtype)
